# revision 1
# baseline (speedup 1.0000x reference)
"""Trainium2 Bass kernel for the e3nn-style point kernel:

    out[z, i, j] = sum_{y,w} Q[i,j,y,w] * Ysh[z,y] * Rad[z,w]      (+ K0 fallback
                                                                     for |r|==0)
    Ysh = real spherical harmonics l=0,1,2 of d = r/|r|  (component norm)
    Rad = relu(|r| * W1 + b1) @ W2 + b2

Strategy (per core, data-parallel over N across 8 cores, NP=8192 points each):
  - Point-major precomputation: radii, Ysh (DVE, packed ops), radial MLP
    hidden h^T via K=1 PE outer-product + ACT relu, Rad point-major via
    PE matmuls with h^T slices as stationary.
  - Khatri-Rao features F[z, (y,w)] = Ysh[z,y]*Rad[z,w] built point-major with
    per-partition-scalar tensor_scalar ops (fp16), plus Ysh appended
    (covers the b2 term through a host-precomputed B[y,ij] = sum_w b2[w]*Q[ij,y,w]).
  - Feature-major F^T obtained with DMA x-bar transposes (fp16, SBUF->SBUF).
  - One big GEMM: out[z, ij] = sum_k F^T[k, z] * Qstack[k, ij] with
    Qstack = [Qmat; B; 0-pad] (host-repacked weights), 7 K-tiles PSUM-accumulated.
"""

import math
from contextlib import ExitStack

import numpy as np

import concourse.bass as bass
import concourse.mybir as mybir
import concourse.tile as tile
from concourse import bacc
from concourse._compat import with_exitstack
from concourse.bass import ds, ts
from concourse.bass_utils import run_bass_kernel_spmd
from concourse.masks import make_identity

F32 = mybir.dt.float32
F16 = mybir.dt.float16
AF = mybir.ActivationFunctionType
OP = mybir.AluOpType

N_TOTAL = 65536
N_CORES = 8
NP = N_TOTAL // N_CORES          # 8192 points per core
NT = NP // 128                   # 64 z-tiles of 128 points
NCH = NT // 4                    # 16 chunks of 4 z-tiles (512 points)
H = 128                          # MLP hidden
W = 96                           # MLP out / radial channels
NY = 9                           # spherical harmonics
IJ = 256                         # 16*16 outputs
KF = 9 * W                       # 864 true feature rows
KV = KF + NY                     # 873 rows incl. Ysh block (for the B term)
KT = 7                           # K-tiles of 128 (896 rows, 873 valid)

SQ3 = math.sqrt(3.0)
SQ5 = math.sqrt(5.0)
SQ15 = math.sqrt(15.0)


@with_exitstack
def _emit(ctx: ExitStack, tc: tile.TileContext, r_ext, q_ext, w1_ext, b1_ext,
          w2_ext, out_ext):
    nc = tc.nc

    consts = ctx.enter_context(tc.tile_pool(name="consts", bufs=1))
    work = ctx.enter_context(tc.tile_pool(name="work", bufs=1))

    # ---------------- constants ----------------
    id128 = consts.tile([128, 128], F32)
    make_identity(nc, id128)

    w1_sb = consts.tile([1, H], F32)
    nc.sync.dma_start(out=w1_sb, in_=w1_ext[:, :])
    b1_sb = consts.tile([H, 1], F32)
    nc.sync.dma_start(out=b1_sb, in_=b1_ext.rearrange("(h o) -> h o", o=1))
    w2_sb = consts.tile([H, W], F16)
    nc.sync.dma_start(out=w2_sb, in_=w2_ext[:, :])

    # Qstack: [128, KT, IJ] fp16, K-tile major (host-prepacked: Q | B | zeros)
    qmat = consts.tile([128, KT, IJ], F16)
    nc.sync.dma_start(out=qmat, in_=q_ext[:, :, :])

    # ---------------- point-major precomputation ----------------
    # r_sb[p, t, c] = r[t*128 + p, c]
    r_sb = work.tile([128, NT, 3], F32)
    nc.sync.dma_start(out=r_sb, in_=r_ext.rearrange("(t p) c -> p t c", p=128))

    rsq = work.tile([128, NT, 3], F32)
    nc.vector.tensor_mul(rsq, r_sb, r_sb)
    rad2 = work.tile([128, NT], F32)
    nc.vector.tensor_reduce(rad2, rsq, axis=mybir.AxisListType.X, op=OP.add)
    radii = work.tile([128, NT], F32)
    nc.scalar.activation(radii, rad2, AF.Sqrt)
    invr = work.tile([128, NT], F32)
    nc.vector.reciprocal(invr, rad2)                    # 1/rad^2
    nc.vector.tensor_mul(invr, invr, radii)             # -> 1/rad

    d = work.tile([128, NT, 3], F32)
    for c in range(3):
        nc.vector.tensor_mul(d[:, :, c], r_sb[:, :, c], invr)
    e = work.tile([128, NT, 3], F32)
    nc.vector.tensor_scalar_mul(e, d, SQ15)
    g = work.tile([128, NT, 3], F32)
    nc.vector.tensor_scalar_mul(g, e, 0.5)

    yw = work.tile([128, NT, NY], F32)
    nc.vector.memset(yw[:, :, 0], 1.0)
    nc.vector.tensor_scalar_mul(yw[:, :, 1], d[:, :, 1], SQ3)
    nc.vector.tensor_scalar_mul(yw[:, :, 2], d[:, :, 2], SQ3)
    nc.vector.tensor_scalar_mul(yw[:, :, 3], d[:, :, 0], SQ3)
    nc.vector.tensor_mul(yw[:, :, 4], e[:, :, 0], d[:, :, 1])
    nc.vector.tensor_mul(yw[:, :, 5], e[:, :, 1], d[:, :, 2])
    t2 = work.tile([128, NT], F32)
    nc.vector.tensor_mul(t2, d[:, :, 2], d[:, :, 2])
    nc.vector.tensor_scalar(yw[:, :, 6], t2, 1.5 * SQ5, -0.5 * SQ5,
                            op0=OP.mult, op1=OP.add)
    nc.vector.tensor_mul(yw[:, :, 7], e[:, :, 0], d[:, :, 2])
    su = work.tile([128, NT], F32)
    sv = work.tile([128, NT], F32)
    nc.vector.tensor_mul(su, g[:, :, 0], d[:, :, 0])
    nc.vector.tensor_mul(sv, g[:, :, 1], d[:, :, 1])
    nc.vector.tensor_sub(yw[:, :, 8], su, sv)

    y16 = work.tile([128, NT, NY], F16)
    nc.vector.tensor_copy(y16, yw)

    # hidden h^T[c, z] = relu(W1[c]*radii[z] + b1[c]), fp16 [128, NP].
    # radii rows are brought to partition 0 via M=1 PE transposes.
    ht = consts.tile([128, NP], F16)
    p_rrow = ctx.enter_context(tc.tile_pool(name="rrow", bufs=2))
    with tc.tile_pool(name="ps_t", bufs=2, space="PSUM") as ps_t, \
         tc.tile_pool(name="ps_h", bufs=2, space="PSUM") as ps_h:
        # dummy transpose so the PE engine observes the Pool semaphore
        # (identity build) before the first real transpose — is_transpose
        # matmuls only support a single sync-wait command in codegen.
        dummy = ps_t.tile([128, 128], F32, tag="dummy")
        nc.tensor.transpose(dummy, id128, id128)
        for ch in range(NCH):
            rps = ps_t.tile([1, 512], F32, tag="rps")
            for j in range(4):
                t = 4 * ch + j
                nc.tensor.transpose(rps[0:1, ds(128 * j, 128)],
                                    radii[:, t:t + 1], id128)
            rrow = p_rrow.tile([1, 512], F32, tag="rrow")
            nc.scalar.copy(rrow, rps)
            hp = ps_h.tile([128, 512], F32, tag="hp")
            nc.tensor.matmul(out=hp, lhsT=w1_sb, rhs=rrow,
                             start=True, stop=True)
            nc.scalar.activation(ht[:, ds(512 * ch, 512)], hp, AF.Relu,
                                 bias=b1_sb, scale=1.0)

    # Rad point-major (no b2): rad_pm[z, t, w], fp16
    rad_pm = consts.tile([128, NT, W], F16)
    with tc.tile_pool(name="ps_r", bufs=2, space="PSUM") as ps_r:
        t = 0
        while t < NT:
            nt = min(5, NT - t)
            rp = ps_r.tile([128, 5 * W], F32, tag="rp")
            for j in range(nt):
                nc.tensor.matmul(out=rp[:, ds(W * j, W)],
                                 lhsT=ht[:, ts(t + j, 128)], rhs=w2_sb,
                                 start=True, stop=True)
            nc.scalar.copy(rad_pm[:, ds(t, nt), :], rp[:, :W * nt])
            t += nt

    # ---------------- main loop ----------------
    p_fpm = ctx.enter_context(tc.tile_pool(name="fpm", bufs=2))
    p_ft = ctx.enter_context(tc.tile_pool(name="ft", bufs=2))
    p_ost = ctx.enter_context(tc.tile_pool(name="ost", bufs=3))
    p_ops = ctx.enter_context(tc.tile_pool(name="ops", bufs=4, space="PSUM"))

    for ch in range(NCH):
        fpm = p_fpm.tile([128, 4, 128 * KT], F16, tag="fpm")
        nc.vector.memset(fpm[:, :, KV:], 0.0)
        for s in range(4):
            t = 4 * ch + s
            nc.vector.tensor_copy(fpm[:, s, 0:W], rad_pm[:, t, :])
            for y in range(1, NY):
                nc.vector.tensor_scalar_mul(fpm[:, s, ds(W * y, W)],
                                            rad_pm[:, t, :],
                                            yw[:, t, y:y + 1])
            nc.vector.tensor_copy(fpm[:, s, KF:KV], y16[:, t, :])

        ft = p_ft.tile([128, KT, 512], F16, tag="ft")
        for s in range(4):
            # one x-bar DMA per subtile: ft[p, k, 128s+j] = fpm[j, s, 128k+p]
            nc.sync.dma_start_transpose(
                out=ft[:, :, ds(128 * s, 128)],
                in_=fpm[:, s, :])

        for j in range(4):
            t = 4 * ch + j
            op = p_ops.tile([128, IJ], F32, tag="op")
            for k in range(KT):
                nc.tensor.matmul(out=op, lhsT=ft[:, k, ts(j, 128)],
                                 rhs=qmat[:, k, :],
                                 start=(k == 0), stop=(k == KT - 1))
            ost = p_ost.tile([128, IJ], F32, tag="ost")
            nc.scalar.copy(ost, op)
            nc.sync.dma_start(out=out_ext[ds(t * 128, 128), :], in_=ost)


def build_nc(repeat: int = 1) -> bass.Bass:
    nc = bacc.Bacc()
    r_ext = nc.declare_dram_parameter("r", [NP, 3], F32, isOutput=False)
    q_ext = nc.declare_dram_parameter("qstack", [128, KT, IJ], F16,
                                      isOutput=False)
    w1_ext = nc.declare_dram_parameter("w1", [1, H], F32, isOutput=False)
    b1_ext = nc.declare_dram_parameter("b1", [H], F32, isOutput=False)
    w2_ext = nc.declare_dram_parameter("w2", [H, W], F16, isOutput=False)
    out_ext = nc.declare_dram_parameter("out", [NP, IJ], F32, isOutput=True)
    with tile.TileContext(nc) as tc:
        for _ in range(repeat):
            _emit(tc, r_ext, q_ext, w1_ext, b1_ext, w2_ext, out_ext)
    nc.compile()
    return nc


def pack_weights(Q, b2):
    """Host-side constant repacking: Qstack = [Qmat; B; 0] in fp16,
    laid out [128, KT, IJ] (K-tile major)."""
    Q = np.asarray(Q, np.float32)
    b2 = np.asarray(b2, np.float32)
    qmat = Q.transpose(2, 3, 0, 1).reshape(KF, IJ)          # [(y,w), (i,j)]
    bmat = np.tensordot(b2, Q, axes=([0], [3]))             # [16,16,9]
    bmat = bmat.transpose(2, 0, 1).reshape(NY, IJ)
    qstack = np.zeros((128 * KT, IJ), np.float16)
    qstack[:KF] = qmat.astype(np.float16)
    qstack[KF:KV] = bmat.astype(np.float16)
    return np.ascontiguousarray(
        qstack.reshape(KT, 128, IJ).transpose(1, 0, 2))


_NC_CACHE = None


def _get_nc():
    global _NC_CACHE
    if _NC_CACHE is None:
        _NC_CACHE = build_nc()
    return _NC_CACHE


def kernel(r, Q, W1, b1, W2, b2, K0):
    r = np.ascontiguousarray(np.asarray(r, dtype=np.float32))
    in_common = {
        "qstack": pack_weights(Q, b2),
        "w1": np.ascontiguousarray(np.asarray(W1, np.float32)),
        "b1": np.ascontiguousarray(np.asarray(b1, np.float32)),
        "w2": np.ascontiguousarray(np.asarray(W2, np.float32).astype(np.float16)),
    }
    in_maps = [dict(r=r[i * NP:(i + 1) * NP], **in_common)
               for i in range(N_CORES)]
    res = run_bass_kernel_spmd(_get_nc(), in_maps, list(range(N_CORES)))
    out = np.concatenate([res.results[i]["out"] for i in range(N_CORES)], 0)
    out = out.reshape(N_TOTAL, 16, 16).astype(np.float32)
    # exact reference semantics for |r| == 0 points (K0 fallback)
    zero = ~(np.linalg.norm(r, axis=1) > 0.0)
    if zero.any():
        out[zero] = np.asarray(K0, np.float32)[None]
    return out



# revision 4
# speedup vs baseline: 1.6213x; 1.6213x over previous
"""Trainium2 Bass kernel for the e3nn-style point kernel:

    out[z, i, j] = sum_{y,w} Q[i,j,y,w] * Ysh[z,y] * Rad[z,w]      (+ K0 fallback
                                                                     for |r|==0)
    Ysh = real spherical harmonics l=0,1,2 of d = r/|r|  (component norm)
    Rad = relu(|r| * W1 + b1) @ W2 + b2

Strategy (per core, data-parallel over N across 8 cores, NP=8192 points each):
  Feature-major construction of the Khatri-Rao features F^T[(y,w), z] --
  no DMA-crossbar transposes (those ran ~1000x slower than modeled on HW):
  - Point-major radii/Ysh on DVE; radii row + Ysh rows go feature-major
    via PE transposes ([128,128] tiles, Ysh padded to 32 y-slots so the
    transposed rows land 32-partition-aligned).
  - hidden h^T via K=1 PE outer product + ACT relu; RadT[w, z] via PE
    (lhsT=W2) directly feature-major.
  - Y broadcast rows Ybc_kt[p, z] = Y[z, y(128kt+p)] via tiny K=9 PE
    matmuls with 0/1 selector matrices (4 concurrent row-groups).
  - Rad replicas rad9[p, slot, z] = Rad[z, w(128kt+p)] via 32-aligned DVE
    copies + SBUF->SBUF DMAs for the partition-shifted segments.
  - F^T = Ybc * rad9 elementwise on DVE (split: some k-tiles via ACT
    PSUM->SBUF copy then 2x-mode fp16 DVE mult, rest direct from PSUM).
  - Main GEMM: out[z, ij] = sum_k F^T[k, z] * Qstack[k, ij], 7 K-tiles
    PSUM-accumulated, Qstack = [Qmat; B; 0] host-prepacked fp16.
  Point order inside a core is block-permuted (z = 64*p + j) so the r load
  is one contiguous DMA; the output DMA un-permutes for free via strides.
"""

import math
from contextlib import ExitStack

import numpy as np

import concourse.bass as bass
import concourse.mybir as mybir
import concourse.tile as tile
from concourse import bacc
from concourse._compat import with_exitstack
from concourse.bass import ds, ts
from concourse.bass_utils import run_bass_kernel_spmd
from concourse.masks import make_identity

F32 = mybir.dt.float32
F16 = mybir.dt.float16
AF = mybir.ActivationFunctionType
OP = mybir.AluOpType

N_TOTAL = 65536
N_CORES = 8
NP = N_TOTAL // N_CORES          # 8192 points per core
NT = NP // 128                   # 64 z-tiles of 128 points
NCH = NT // 4                    # 16 chunks of 4 z-tiles (512 points)
H = 128                          # MLP hidden
W = 96                           # MLP out / radial channels
NY = 9                           # spherical harmonics
NYP = 32                         # padded y-slots (transpose alignment)
IJ = 256                         # 16*16 outputs
KF = 9 * W                       # 864 true feature rows
KV = KF + NY                     # 873 rows incl. Ysh block (for the B term)
KT = 7                           # K-tiles of 128 (896 rows, 873 valid)

SQ3 = math.sqrt(3.0)
SQ5 = math.sqrt(5.0)
SQ15 = math.sqrt(15.0)

# rad9 slot layout: slot s holds rows p -> radt[w] per the (y,w) k-tile maps.
# k-tiles 0..5 repeat with period 3 (128*3 == 96*4); k-tile 6 is slot 3.
RAD9_SLOT = [0, 1, 2, 0, 1, 2, 3]
# (out_base, in_base, nrows) with partition moves decomposed into 32-aligned
# pieces (DVE cross-partition moves are only legal quadrant-aligned <=32 rows;
# shift-free segments can be any size).
RAD9_DVE = [
    (0, 0, 0, 96),      # slot 0: w = 0..95 at p 0..95 (no shift)
    (3, 0, 0, 96),      # slot 3: w = 0..95 at p 0..95 (no shift)
]
RAD9_DMA = [
    (0, 96, 0, 32),     # slot 0: p 96..127  <- w 0..31
    (1, 0, 32, 64),     # slot 1: p 0..63    <- w 32..95
    (1, 64, 0, 64),     # slot 1: p 64..127  <- w 0..63
    (2, 0, 64, 32),     # slot 2: p 0..31    <- w 64..95
    (2, 32, 0, 96),     # slot 2: p 32..127  <- w 0..95
]
# k-tiles whose F-mult goes via ACT copy to SBUF + 2x fp16 DVE mult
ACT_KTS = (0, 1, 2)


@with_exitstack
def _emit(ctx: ExitStack, tc: tile.TileContext, r_ext, q_ext, mks_ext, w1_ext,
          b1_ext, w2_ext, out_ext):
    nc = tc.nc

    consts = ctx.enter_context(tc.tile_pool(name="consts", bufs=1))
    persist = ctx.enter_context(tc.tile_pool(name="persist", bufs=1))

    # ---------------- constants ----------------
    id128 = consts.tile([128, 128], F32)
    make_identity(nc, id128)

    w1_sb = consts.tile([1, H], F32)
    nc.sync.dma_start(out=w1_sb, in_=w1_ext[:, :])
    b1_sb = consts.tile([H, 1], F32)
    nc.sync.dma_start(out=b1_sb, in_=b1_ext.rearrange("(h o) -> h o", o=1))
    w2_sb = consts.tile([H, W], F16)
    nc.sync.dma_start(out=w2_sb, in_=w2_ext[:, :])
    qmat = consts.tile([128, KT, IJ], F16)
    nc.sync.dma_start(out=qmat, in_=q_ext[:, :, :])
    mks = consts.tile([128, KT, 128], F16)
    nc.sync.dma_start(out=mks, in_=mks_ext[:, :, :])

    # persistent feature-major tensors
    ht = persist.tile([128, NP], F16)       # hidden h^T
    radt = persist.tile([W, NP], F16)       # Rad^T (no b2)
    y4 = persist.tile([128, NP], F16)       # Ysh^T rows at bases 0/32/64/96
    rad9 = persist.tile([128, 4, NP], F16)  # Rad rows in k-tile layout

    # ---------------- point-major precomputation ----------------
    # Block point order: z = 64*p + j  (one contiguous DMA for r)
    prep = ctx.enter_context(tc.tile_pool(name="prep", bufs=1))
    r_sb = prep.tile([128, NT, 3], F32)
    nc.sync.dma_start(out=r_sb, in_=r_ext.rearrange("(p q) c -> p q c", p=128))

    rsq = prep.tile([128, NT, 3], F32)
    nc.vector.tensor_mul(rsq, r_sb, r_sb)
    rad2 = prep.tile([128, NT], F32)
    nc.vector.tensor_reduce(rad2, rsq, axis=mybir.AxisListType.X, op=OP.add)
    radii = prep.tile([128, NT], F32)
    nc.scalar.activation(radii, rad2, AF.Sqrt)
    invr = prep.tile([128, NT], F32)
    nc.vector.reciprocal(invr, rad2)                    # 1/rad^2
    nc.vector.tensor_mul(invr, invr, radii)             # -> 1/rad

    d = prep.tile([128, NT, 3], F32)
    for c in range(3):
        nc.vector.tensor_mul(d[:, :, c], r_sb[:, :, c], invr)
    e = prep.tile([128, NT, 3], F32)
    nc.vector.tensor_scalar_mul(e, d, SQ15)
    g = prep.tile([128, NT, 3], F32)
    nc.vector.tensor_scalar_mul(g, e, 0.5)

    # Ysh point-major, padded to 32 y-slots so PE transposes land 32-aligned
    yw = prep.tile([128, NT, NYP], F32)
    nc.vector.memset(yw[:, :, NY:], 0.0)
    nc.vector.memset(yw[:, :, 0], 1.0)
    nc.vector.tensor_scalar_mul(yw[:, :, 1], d[:, :, 1], SQ3)
    nc.vector.tensor_scalar_mul(yw[:, :, 2], d[:, :, 2], SQ3)
    nc.vector.tensor_scalar_mul(yw[:, :, 3], d[:, :, 0], SQ3)
    nc.vector.tensor_mul(yw[:, :, 4], e[:, :, 0], d[:, :, 1])
    nc.vector.tensor_mul(yw[:, :, 5], e[:, :, 1], d[:, :, 2])
    t2 = prep.tile([128, NT], F32)
    nc.vector.tensor_mul(t2, d[:, :, 2], d[:, :, 2])
    nc.vector.tensor_scalar(yw[:, :, 6], t2, 1.5 * SQ5, -0.5 * SQ5,
                            op0=OP.mult, op1=OP.add)
    nc.vector.tensor_mul(yw[:, :, 7], e[:, :, 0], d[:, :, 2])
    su = prep.tile([128, NT], F32)
    sv = prep.tile([128, NT], F32)
    nc.vector.tensor_mul(su, g[:, :, 0], d[:, :, 0])
    nc.vector.tensor_mul(sv, g[:, :, 1], d[:, :, 1])
    nc.vector.tensor_sub(yw[:, :, 8], su, sv)

    # ---------------- feature-major precomputation ----------------
    with tc.tile_pool(name="ps_t", bufs=1, space="PSUM") as ps_t, \
         tc.tile_pool(name="ps_y", bufs=2, space="PSUM") as ps_y, \
         tc.tile_pool(name="ps_h", bufs=2, space="PSUM") as ps_h, \
         tc.tile_pool(name="ps_r", bufs=2, space="PSUM") as ps_r, \
         tc.tile_pool(name="rw", bufs=2) as p_rw:
        # dummy transpose so the PE engine observes the identity-build
        # semaphore before the real transposes (is_transpose matmuls only
        # support a single sync-wait command in codegen).
        dummy = ps_y.tile([128, 128], F32, tag="yt")
        nc.tensor.transpose(dummy, id128, id128)

        # radii row: [128, 64] -T-> [64, 128] -> SBUF -> flatten to [1, NP]
        rt_ps = ps_t.tile([NT, 128], F32, tag="rt")
        nc.tensor.transpose(rt_ps, radii, id128)
        rts = p_rw.tile([NT, 128], F32, tag="rts")
        nc.scalar.copy(rts, rt_ps)
        rrow = prep.tile([1, NP], F32)
        nc.sync.dma_start(out=rrow[0:1, :], in_=rts[:, :])

        # hidden h^T[h, z] = relu(W1[h]*r[z] + b1[h])
        for c in range(NCH):
            hp = ps_h.tile([128, 512], F32, tag="hp")
            nc.tensor.matmul(out=hp, lhsT=w1_sb, rhs=rrow[0:1, ts(c, 512)],
                             start=True, stop=True)
            nc.scalar.activation(ht[:, ts(c, 512)], hp, AF.Relu,
                                 bias=b1_sb, scale=1.0)

        # Rad^T[w, z] = sum_h W2[h, w] h^T[h, z]
        for c in range(NCH):
            rp = ps_r.tile([W, 512], F32, tag="rp")
            nc.tensor.matmul(out=rp, lhsT=w2_sb, rhs=ht[:, ts(c, 512)],
                             start=True, stop=True)
            nc.scalar.copy(radt[:, ts(c, 512)], rp)

        # Ysh^T rows: per chunk transpose [128, 4*32] -> [4*32, 128];
        # rows 32t..32t+8 hold Y[z=...,y] for z-tile 4c+t.
        for c in range(NCH):
            yt_ps = ps_y.tile([128, 128], F32, tag="yt")
            nc.tensor.transpose(yt_ps, yw[:, ds(4 * c, 4), :], id128)
            for t in range(4):
                nc.vector.tensor_copy(y4[0:NY, ds(512 * c + 128 * t, 128)],
                                      yt_ps[ds(32 * t, NY), :])

    # replicate Ysh^T rows to partition bases 32/64/96 (row-tiled Ybc MMs)
    for gb in (32, 64, 96):
        nc.vector.tensor_copy(y4[gb:gb + NY, :], y4[0:NY, :])

    # rad9: Rad rows in k-tile partition layout (+1.0 for the B-block rows)
    for s, p0, w0, L in RAD9_DVE:
        nc.vector.tensor_copy(rad9[p0:p0 + L, s, :], radt[w0:w0 + L, :])
    for s, p0, w0, L in RAD9_DMA:
        nc.sync.dma_start(out=rad9[p0:p0 + L, s, :], in_=radt[w0:w0 + L, :])
    nc.vector.memset(rad9[96:128, 3, :], 1.0)

    # ---------------- main loop ----------------
    p_ft = ctx.enter_context(tc.tile_pool(name="ft", bufs=2))
    p_ybs = ctx.enter_context(tc.tile_pool(name="ybs", bufs=3))
    p_ost = ctx.enter_context(tc.tile_pool(name="ost", bufs=3))
    p_ybc = ctx.enter_context(tc.tile_pool(name="ybc", bufs=4, space="PSUM"))
    p_ops = ctx.enter_context(tc.tile_pool(name="ops", bufs=3, space="PSUM"))

    out_v = out_ext.rearrange("(p t) i -> p t i", t=NT)
    for c in range(NCH):
        ft = p_ft.tile([128, KT, 512], F16, tag="ft")
        for kt in range(KT):
            gb = 32 * (kt % 4)
            ybc = p_ybc.tile([128, 512], F32, tag="ybc")
            nc.tensor.matmul(out=ybc, lhsT=mks[gb:gb + NY, kt, :],
                             rhs=y4[gb:gb + NY, ts(c, 512)],
                             start=True, stop=True, tile_position=(gb, 0))
            rad_s = rad9[:, RAD9_SLOT[kt], ts(c, 512)]
            if kt in ACT_KTS:
                ybs = p_ybs.tile([128, 512], F16, tag="ybs")
                nc.scalar.copy(ybs, ybc)
                nc.vector.tensor_mul(ft[:, kt, :], ybs, rad_s)
            else:
                nc.vector.tensor_mul(ft[:, kt, :], ybc, rad_s)

        for j in range(4):
            t = 4 * c + j
            op = p_ops.tile([128, IJ], F32, tag="op")
            for kt in range(KT):
                nc.tensor.matmul(out=op, lhsT=ft[:, kt, ts(j, 128)],
                                 rhs=qmat[:, kt, :],
                                 start=(kt == 0), stop=(kt == KT - 1))
            ost = p_ost.tile([128, IJ], F32, tag="ost")
            nc.scalar.copy(ost, op)
            # z = 64*p + t: partition p writes row 64p + t
            nc.sync.dma_start(out=out_v[:, t, :], in_=ost)


def build_nc(repeat: int = 1) -> bass.Bass:
    nc = bacc.Bacc()
    r_ext = nc.declare_dram_parameter("r", [NP, 3], F32, isOutput=False)
    q_ext = nc.declare_dram_parameter("qstack", [128, KT, IJ], F16,
                                      isOutput=False)
    mks_ext = nc.declare_dram_parameter("mks", [128, KT, 128], F16,
                                        isOutput=False)
    w1_ext = nc.declare_dram_parameter("w1", [1, H], F32, isOutput=False)
    b1_ext = nc.declare_dram_parameter("b1", [H], F32, isOutput=False)
    w2_ext = nc.declare_dram_parameter("w2", [H, W], F16, isOutput=False)
    out_ext = nc.declare_dram_parameter("out", [NP, IJ], F32, isOutput=True)
    with tile.TileContext(nc) as tc:
        for _ in range(repeat):
            _emit(tc, r_ext, q_ext, mks_ext, w1_ext, b1_ext, w2_ext, out_ext)
    nc.compile()
    return nc


def pack_weights(Q, b2):
    """Host-side constant repacking: Qstack = [Qmat; B; 0] in fp16,
    laid out [128, KT, IJ] (K-tile major)."""
    Q = np.asarray(Q, np.float32)
    b2 = np.asarray(b2, np.float32)
    qmat = Q.transpose(2, 3, 0, 1).reshape(KF, IJ)          # [(y,w), (i,j)]
    bmat = np.tensordot(b2, Q, axes=([0], [3]))             # [16,16,9]
    bmat = bmat.transpose(2, 0, 1).reshape(NY, IJ)
    qstack = np.zeros((128 * KT, IJ), np.float16)
    qstack[:KF] = qmat.astype(np.float16)
    qstack[KF:KV] = bmat.astype(np.float16)
    return np.ascontiguousarray(
        qstack.reshape(KT, 128, IJ).transpose(1, 0, 2))


def pack_mks():
    """Ybc selector: mks[32g + y, kt, p] = 1 iff feature row 128kt+p uses
    Ysh component y (replicated at 4 partition bases for PE row-tiling)."""
    mks = np.zeros((128, KT, 128), np.float16)
    for kt in range(KT):
        for p in range(128):
            k = 128 * kt + p
            if k < KF:
                y = k // W
            elif k < KV:
                y = k - KF
            else:
                continue
            for gb in range(4):
                mks[32 * gb + y, kt, p] = 1.0
    return np.ascontiguousarray(mks)


_NC_CACHE = None


def _get_nc():
    global _NC_CACHE
    if _NC_CACHE is None:
        _NC_CACHE = build_nc()
    return _NC_CACHE


def kernel(r, Q, W1, b1, W2, b2, K0):
    r = np.ascontiguousarray(np.asarray(r, dtype=np.float32))
    in_common = {
        "qstack": pack_weights(Q, b2),
        "mks": pack_mks(),
        "w1": np.ascontiguousarray(np.asarray(W1, np.float32)),
        "b1": np.ascontiguousarray(np.asarray(b1, np.float32)),
        "w2": np.ascontiguousarray(np.asarray(W2, np.float32).astype(np.float16)),
    }
    in_maps = [dict(r=r[i * NP:(i + 1) * NP], **in_common)
               for i in range(N_CORES)]
    res = run_bass_kernel_spmd(_get_nc(), in_maps, list(range(N_CORES)))
    out = np.concatenate([res.results[i]["out"] for i in range(N_CORES)], 0)
    out = out.reshape(N_TOTAL, 16, 16).astype(np.float32)
    # exact reference semantics for |r| == 0 points (K0 fallback)
    zero = ~(np.linalg.norm(r, axis=1) > 0.0)
    if zero.any():
        out[zero] = np.asarray(K0, np.float32)[None]
    return out
